# revision 7
# baseline (speedup 1.0000x reference)
"""BCH/RS systematic encoder kernel for Trainium2 (8 NeuronCores, data parallel).

Computes out = concat([msg, (msg @ Gp) mod 2], axis=-1) for
msg [16384, 1000] f32 of 0/1 bits and Gp [1000, 256] f32 of 0/1 bits.

v2 design (per core, 2048 rows, 8 superchunks of 2x128 rows), fixing the
v1 serialization (all loads upfront on one SWDGE queue -> first load done
at ~20us; stores on the same queue; DVE-ordinal semaphores chaining each
superchunk's transpose behind the previous superchunk's DVE ops):

  - HWDGE f32 load (sync ring) msg chunk -> a [128, 2, 1000] f32, paced
    at prefetch depth 3 by interleaving loads/stores in the sync FIFO
  - HWDGE f32 msg-store (sync ring) a -> out[:, :1000] as soon as the
    load lands (copy-through never waits on compute)
  - DVE cast a -> abf bf16 (0/1 exact), pad cols 1000:1024 memset
  - ONE xbar transpose per superchunk on the scalar (ACT) HWDGE ring:
    abf [128, 2048] -> b [128, 16, 128] (16 independent 128x128 tiles)
  - 16 accumulating bf16 matmuls: acc[m, 2*256] += b_k.T @ Gp_k
  - parity mod-2 on DVE only (ISA has no f32 mod): CAST psum f32->i32,
    AND 1, CAST i32->f32 (all exact for integer sums <= 1000)
  - SWDGE f32 parity-store (gpsimd ring) par -> out[:, 1000:1256]

Per-engine emission order is software-pipelined so the Tile scheduler's
engine-ordinal counting semaphores never chain superchunk N's transpose
behind superchunk N-1's compute: cast(it+2) is emitted BEFORE the parity
chain of superchunk it on the DVE stream, and the ACT ring carries ONLY
transposes.

HBM traffic/core = 8.19 MB read + 10.29 MB write (the minimum) at
~358 GB/s per core -> ~52 us floor.
"""

import os
import sys

import numpy as np

if os.path.isdir("/opt/trn_rl_repo") and "/opt/trn_rl_repo" not in sys.path:
    sys.path.insert(0, "/opt/trn_rl_repo")

import ml_dtypes

import concourse.bacc as bacc
import concourse.mybir as mybir
import concourse.tile as tile
from concourse.bass_utils import run_bass_kernel_spmd

BATCH = 16384
MSG = 1000
NPAR = 256
NCORES = 8
ROWS = BATCH // NCORES  # 2048
P = 128
KCH = 8  # k chunks; padded K = 1024
KPAD = KCH * P
SC = 2  # m-chunks per superchunk
PF = 3  # load prefetch depth (superchunks)

# test.py pokes these for profiling
TRACE = False
LAST_RESULT = None

_CACHE = {}


def build_nc(rows=ROWS):
    """Emit the Bass/Tile IR for one core handling `rows` rows."""
    n_it = rows // (SC * P)
    pf = min(PF, n_it)
    nc = bacc.Bacc("TRN2", target_bir_lowering=False, debug=False)
    msg = nc.dram_tensor("msg", [rows, MSG], mybir.dt.float32, kind="ExternalInput")
    gp = nc.dram_tensor("gp", [P, KCH * NPAR], mybir.dt.bfloat16, kind="ExternalInput")
    out = nc.dram_tensor(
        "out", [rows, MSG + NPAR], mybir.dt.float32, kind="ExternalOutput"
    )

    msg3 = msg[:, :].rearrange("(s c p) k -> s c p k", c=SC, p=P)
    out3 = out[:, :].rearrange("(s c p) k -> s c p k", c=SC, p=P)

    with tile.TileContext(nc) as tc:
        with (
            tc.tile_pool(name="gpool", bufs=1) as gpool,
            tc.tile_pool(name="apool", bufs=pf + 2) as apool,
            tc.tile_pool(name="bfpool", bufs=4) as bfpool,
            tc.tile_pool(name="bpool", bufs=4) as bpool,
            tc.tile_pool(name="parpool", bufs=3) as parpool,
            tc.tile_pool(name="ipool", bufs=3) as ipool,
            tc.tile_pool(name="ppool", bufs=4, space="PSUM") as ppool,
        ):
            # Gp resident in SBUF: gsb[q, kb*256 + n] = Gp_padded[kb*128 + q, n]
            gsb = gpool.tile([P, KCH * NPAR], mybir.dt.bfloat16)
            nc.sync.dma_start(out=gsb[:, :], in_=gp[:, :])

            a_t = {}
            abf_t = {}
            b_t = {}
            acc_t = {}

            def emit_load(it):
                a = apool.tile([P, SC, MSG], mybir.dt.float32, tag="a")
                nc.sync.dma_start(
                    out=a[:, :, :], in_=msg3[it, :, :, :].rearrange("c p k -> p c k")
                )
                a_t[it] = a

            def emit_msg_store(it):
                nc.sync.dma_start(
                    out=out3[it, :, :, 0:MSG].rearrange("c p k -> p c k"),
                    in_=a_t.pop(it)[:, :, :],
                )

            def emit_cast(it):
                abf = bfpool.tile([P, SC, KPAD], mybir.dt.bfloat16, tag="abf")
                nc.vector.memset(abf[:, :, MSG:KPAD], 0)
                nc.vector.tensor_copy(abf[:, :, 0:MSG], a_t[it][:, :, :])
                abf_t[it] = abf

            def emit_tr(it):
                # one xbar transpose for the whole superchunk, on the ACT
                # HWDGE ring (all TRs on ONE ring: concurrent xbar
                # transposes from two rings corrupt each other)
                b = bpool.tile([P, SC * KCH, P], mybir.dt.bfloat16, tag="b")
                nc.scalar.dma_start(
                    out=b[:, :, :],
                    in_=abf_t.pop(it)[:, :, :].rearrange("p c k -> p (c k)"),
                    transpose=True,
                )
                b_t[it] = b

            def emit_mm(it):
                b = b_t.pop(it)
                acc = ppool.tile([P, SC * NPAR], mybir.dt.float32, tag="acc")
                for c in range(SC):
                    for kb in range(KCH):
                        kk = P if kb < KCH - 1 else MSG - (KCH - 1) * P  # 104 tail
                        nc.tensor.matmul(
                            acc[:, c * NPAR : (c + 1) * NPAR],
                            b[0:kk, c * KCH + kb, :],
                            gsb[0:kk, kb * NPAR : (kb + 1) * NPAR],
                            start=(kb == 0),
                            stop=(kb == KCH - 1),
                        )
                acc_t[it] = acc

            def emit_parity(it):
                # mod 2 on DVE only: psum f32 -> i32 (numeric cast, exact
                # for integer sums <= 1000), AND 1, i32 -> f32
                ci = ipool.tile([P, SC * NPAR], mybir.dt.int32, tag="ci")
                nc.vector.tensor_copy(ci[:, :], acc_t.pop(it)[:, :])
                nc.vector.tensor_scalar(
                    ci[:, :], ci[:, :], 1, None, mybir.AluOpType.bitwise_and
                )
                par = parpool.tile([P, SC, NPAR], mybir.dt.float32, tag="par")
                nc.vector.tensor_copy(
                    par[:, :, :].rearrange("p c n -> p (c n)"), ci[:, :]
                )
                return par

            def emit_par_store(it, par):
                nc.gpsimd.dma_start(
                    out=out3[it, :, :, MSG : MSG + NPAR].rearrange("c p k -> p c k"),
                    in_=par[:, :, :],
                )

            for i in range(pf):
                emit_load(i)
            emit_cast(0)
            if n_it > 1:
                emit_cast(1)
            emit_tr(0)
            for it in range(n_it):
                if it + pf < n_it:
                    emit_load(it + pf)
                emit_msg_store(it)
                if it + 2 < n_it:
                    emit_cast(it + 2)
                if it + 1 < n_it:
                    emit_tr(it + 1)
                emit_mm(it)
                par = emit_parity(it)
                emit_par_store(it, par)

    nc.compile()
    return nc


def prep_gp(Gp):
    """Pad Gp to 1024 rows and swizzle to the [128, 8*256] bf16 SBUF layout."""
    gp = np.asarray(Gp, dtype=np.float32)
    gp_pad = np.zeros((KPAD, NPAR), dtype=np.float32)
    gp_pad[:MSG] = gp
    gsw = gp_pad.reshape(KCH, P, NPAR).transpose(1, 0, 2).reshape(P, KCH * NPAR)
    return np.ascontiguousarray(gsw).astype(ml_dtypes.bfloat16)


def kernel(message_bits, Gp):
    global LAST_RESULT
    msg = np.ascontiguousarray(np.asarray(message_bits, dtype=np.float32))
    assert msg.shape == (BATCH, MSG), msg.shape
    gsw = prep_gp(Gp)

    if "nc" not in _CACHE:
        _CACHE["nc"] = build_nc()
    nc = _CACHE["nc"]

    in_maps = [
        {"msg": msg[i * ROWS : (i + 1) * ROWS], "gp": gsw} for i in range(NCORES)
    ]
    res = run_bass_kernel_spmd(
        nc, in_maps, core_ids=list(range(NCORES)), trace=TRACE
    )
    LAST_RESULT = res
    return np.concatenate([r["out"] for r in res.results], axis=0)


# revision 8
# speedup vs baseline: 1.0628x; 1.0628x over previous
"""BCH/RS systematic encoder kernel for Trainium2 (8 NeuronCores, data parallel).

Computes out = concat([msg, (msg @ Gp) mod 2], axis=-1) for
msg [16384, 1000] f32 of 0/1 bits and Gp [1000, 256] f32 of 0/1 bits.

v2 design (per core, 2048 rows, 8 superchunks of 2x128 rows), fixing the
v1 serialization (all loads upfront on one SWDGE queue -> first load done
at ~20us; stores on the same queue; DVE-ordinal semaphores chaining each
superchunk's transpose behind the previous superchunk's DVE ops):

  - HWDGE f32 load (sync ring) msg chunk -> a [128, 2, 1000] f32, paced
    at prefetch depth 3 by interleaving loads/stores in the sync FIFO
  - HWDGE f32 msg-store (sync ring) a -> out[:, :1000] as soon as the
    load lands (copy-through never waits on compute)
  - DVE cast a -> abf bf16 (0/1 exact), pad cols 1000:1024 memset
  - ONE xbar transpose per superchunk on the scalar (ACT) HWDGE ring:
    abf [128, 2048] -> b [128, 16, 128] (16 independent 128x128 tiles)
  - 16 accumulating bf16 matmuls: acc[m, 2*256] += b_k.T @ Gp_k
  - parity mod-2 on DVE only (ISA has no f32 mod): CAST psum f32->i32,
    AND 1, CAST i32->f32 (all exact for integer sums <= 1000)
  - SWDGE f32 parity-store (gpsimd ring) par -> out[:, 1000:1256]

Per-engine emission order is software-pipelined so the Tile scheduler's
engine-ordinal counting semaphores never chain superchunk N's transpose
behind superchunk N-1's compute: cast(it+2) is emitted BEFORE the parity
chain of superchunk it on the DVE stream, and the ACT ring carries ONLY
transposes.

HBM traffic/core = 8.19 MB read + 10.29 MB write (the minimum) at
~358 GB/s per core -> ~52 us floor.
"""

import os
import sys

import numpy as np

if os.path.isdir("/opt/trn_rl_repo") and "/opt/trn_rl_repo" not in sys.path:
    sys.path.insert(0, "/opt/trn_rl_repo")

import ml_dtypes

import concourse.bacc as bacc
import concourse.mybir as mybir
import concourse.tile as tile
from concourse.bass_utils import run_bass_kernel_spmd

BATCH = 16384
MSG = 1000
NPAR = 256
NCORES = 8
ROWS = BATCH // NCORES  # 2048
P = 128
KCH = 8  # k chunks; padded K = 1024
KPAD = KCH * P
SC = 2  # m-chunks per superchunk
PF = 3  # load prefetch depth (superchunks)

# test.py pokes these for profiling
TRACE = False
LAST_RESULT = None

_CACHE = {}


def build_nc(rows=ROWS):
    """Emit the Bass/Tile IR for one core handling `rows` rows."""
    n_it = rows // (SC * P)
    pf = min(PF, n_it)
    nc = bacc.Bacc("TRN2", target_bir_lowering=False, debug=False)
    msg = nc.dram_tensor("msg", [rows, MSG], mybir.dt.float32, kind="ExternalInput")
    gp = nc.dram_tensor("gp", [P, KCH * NPAR], mybir.dt.bfloat16, kind="ExternalInput")
    out = nc.dram_tensor(
        "out", [rows, MSG + NPAR], mybir.dt.float32, kind="ExternalOutput"
    )

    msg3 = msg[:, :].rearrange("(s c p) k -> s c p k", c=SC, p=P)
    out3 = out[:, :].rearrange("(s c p) k -> s c p k", c=SC, p=P)

    with tile.TileContext(nc) as tc:
        with (
            # every buffer's full live range fits in its pool, so the Tile
            # scheduler never emits slot-release relay waits (those ride
            # engine queues and serialize the pipeline cross-superchunk)
            tc.tile_pool(name="gpool", bufs=1) as gpool,
            tc.tile_pool(name="apool", bufs=n_it + 1) as apool,
            tc.tile_pool(name="bfpool", bufs=4) as bfpool,
            tc.tile_pool(name="bpool", bufs=4) as bpool,
            tc.tile_pool(name="parpool", bufs=n_it) as parpool,
            tc.tile_pool(name="ipool", bufs=4) as ipool,
            tc.tile_pool(name="ppool", bufs=8, space="PSUM") as ppool,
        ):
            # Gp resident in SBUF: gsb[q, kb*256 + n] = Gp_padded[kb*128 + q, n]
            # (loaded on the otherwise-idle SWDGE queue so the sync HWDGE
            # ring starts draining load(0) immediately)
            gsb = gpool.tile([P, KCH * NPAR], mybir.dt.bfloat16)
            nc.gpsimd.dma_start(out=gsb[:, :], in_=gp[:, :])

            a_t = {}
            abf_t = {}
            b_t = {}
            acc_t = {}

            def emit_load(it):
                a = apool.tile([P, SC, MSG], mybir.dt.float32, tag="a")
                nc.sync.dma_start(
                    out=a[:, :, :], in_=msg3[it, :, :, :].rearrange("c p k -> p c k")
                )
                a_t[it] = a

            def emit_msg_store(it):
                nc.sync.dma_start(
                    out=out3[it, :, :, 0:MSG].rearrange("c p k -> p c k"),
                    in_=a_t.pop(it)[:, :, :],
                )

            def emit_cast(it):
                abf = bfpool.tile([P, SC, KPAD], mybir.dt.bfloat16, tag="abf")
                nc.vector.memset(abf[:, :, MSG:KPAD], 0)
                nc.vector.tensor_copy(abf[:, :, 0:MSG], a_t[it][:, :, :])
                abf_t[it] = abf

            def emit_tr(it):
                # one xbar transpose for the whole superchunk, on the ACT
                # HWDGE ring (all TRs on ONE ring: concurrent xbar
                # transposes from two rings corrupt each other)
                b = bpool.tile([P, SC * KCH, P], mybir.dt.bfloat16, tag="b")
                nc.scalar.dma_start(
                    out=b[:, :, :],
                    in_=abf_t.pop(it)[:, :, :].rearrange("p c k -> p (c k)"),
                    transpose=True,
                )
                b_t[it] = b

            def emit_mm(it):
                b = b_t.pop(it)
                acc = ppool.tile([P, SC * NPAR], mybir.dt.float32, tag="acc")
                for c in range(SC):
                    for kb in range(KCH):
                        kk = P if kb < KCH - 1 else MSG - (KCH - 1) * P  # 104 tail
                        nc.tensor.matmul(
                            acc[:, c * NPAR : (c + 1) * NPAR],
                            b[0:kk, c * KCH + kb, :],
                            gsb[0:kk, kb * NPAR : (kb + 1) * NPAR],
                            start=(kb == 0),
                            stop=(kb == KCH - 1),
                        )
                acc_t[it] = acc

            def emit_parity(it):
                # mod 2 on DVE only: psum f32 -> i32 (numeric cast, exact
                # for integer sums <= 1000), AND 1, i32 -> f32
                ci = ipool.tile([P, SC * NPAR], mybir.dt.int32, tag="ci")
                nc.vector.tensor_copy(ci[:, :], acc_t.pop(it)[:, :])
                nc.vector.tensor_scalar(
                    ci[:, :], ci[:, :], 1, None, mybir.AluOpType.bitwise_and
                )
                par = parpool.tile([P, SC, NPAR], mybir.dt.float32, tag="par")
                nc.vector.tensor_copy(
                    par[:, :, :].rearrange("p c n -> p (c n)"), ci[:, :]
                )
                return par

            def emit_par_store(it, par):
                nc.gpsimd.dma_start(
                    out=out3[it, :, :, MSG : MSG + NPAR].rearrange("c p k -> p c k"),
                    in_=par[:, :, :],
                )

            for i in range(pf):
                emit_load(i)
            emit_cast(0)
            if n_it > 1:
                emit_cast(1)
            emit_tr(0)
            for it in range(n_it):
                if it + pf < n_it:
                    emit_load(it + pf)
                emit_msg_store(it)
                if it + 2 < n_it:
                    emit_cast(it + 2)
                if it + 1 < n_it:
                    emit_tr(it + 1)
                emit_mm(it)
                par = emit_parity(it)
                emit_par_store(it, par)

    nc.compile()
    return nc


def prep_gp(Gp):
    """Pad Gp to 1024 rows and swizzle to the [128, 8*256] bf16 SBUF layout."""
    gp = np.asarray(Gp, dtype=np.float32)
    gp_pad = np.zeros((KPAD, NPAR), dtype=np.float32)
    gp_pad[:MSG] = gp
    gsw = gp_pad.reshape(KCH, P, NPAR).transpose(1, 0, 2).reshape(P, KCH * NPAR)
    return np.ascontiguousarray(gsw).astype(ml_dtypes.bfloat16)


def kernel(message_bits, Gp):
    global LAST_RESULT
    msg = np.ascontiguousarray(np.asarray(message_bits, dtype=np.float32))
    assert msg.shape == (BATCH, MSG), msg.shape
    gsw = prep_gp(Gp)

    if "nc" not in _CACHE:
        _CACHE["nc"] = build_nc()
    nc = _CACHE["nc"]

    in_maps = [
        {"msg": msg[i * ROWS : (i + 1) * ROWS], "gp": gsw} for i in range(NCORES)
    ]
    res = run_bass_kernel_spmd(
        nc, in_maps, core_ids=list(range(NCORES)), trace=TRACE
    )
    LAST_RESULT = res
    return np.concatenate([r["out"] for r in res.results], axis=0)


# revision 9
# speedup vs baseline: 1.1986x; 1.1278x over previous
"""BCH/RS systematic encoder kernel for Trainium2 (8 NeuronCores, data parallel).

Computes out = concat([msg, (msg @ Gp) mod 2], axis=-1) for
msg [16384, 1000] f32 of 0/1 bits and Gp [1000, 256] f32 of 0/1 bits.

v2 design (per core, 2048 rows, 8 superchunks of 2x128 rows), fixing the
v1 serialization (all loads upfront on one SWDGE queue -> first load done
at ~20us; stores on the same queue; DVE-ordinal semaphores chaining each
superchunk's transpose behind the previous superchunk's DVE ops):

  - HWDGE f32 load (sync ring) msg chunk -> a [128, 2, 1000] f32, paced
    at prefetch depth 3 by interleaving loads/stores in the sync FIFO
  - HWDGE f32 msg-store (sync ring) a -> out[:, :1000] as soon as the
    load lands (copy-through never waits on compute)
  - DVE cast a -> abf bf16 (0/1 exact), pad cols 1000:1024 memset
  - ONE xbar transpose per superchunk on the scalar (ACT) HWDGE ring:
    abf [128, 2048] -> b [128, 16, 128] (16 independent 128x128 tiles)
  - 16 accumulating bf16 matmuls: acc[m, 2*256] += b_k.T @ Gp_k
  - parity mod-2 on DVE only (ISA has no f32 mod): CAST psum f32->i32,
    AND 1, CAST i32->f32 (all exact for integer sums <= 1000)
  - SWDGE f32 parity-store (gpsimd ring) par -> out[:, 1000:1256]

Per-engine emission order is software-pipelined so the Tile scheduler's
engine-ordinal counting semaphores never chain superchunk N's transpose
behind superchunk N-1's compute: cast(it+2) is emitted BEFORE the parity
chain of superchunk it on the DVE stream, and the ACT ring carries ONLY
transposes.

HBM traffic/core = 8.19 MB read + 10.29 MB write (the minimum) at
~358 GB/s per core -> ~52 us floor.
"""

import os
import sys

import numpy as np

if os.path.isdir("/opt/trn_rl_repo") and "/opt/trn_rl_repo" not in sys.path:
    sys.path.insert(0, "/opt/trn_rl_repo")

import ml_dtypes

import concourse.bacc as bacc
import concourse.mybir as mybir
import concourse.tile as tile
from concourse.bass_utils import run_bass_kernel_spmd

BATCH = 16384
MSG = 1000
NPAR = 256
NCORES = 8
ROWS = BATCH // NCORES  # 2048
P = 128
KCH = 8  # k chunks; padded K = 1024
KPAD = KCH * P
SC = 2  # m-chunks per superchunk
# all loads are emitted upfront: the sync HWDGE ring drains them in order
# (load it completes at ~2.9us*(it+1)), and the first 8 HWDGE sem lanes
# are consumed by pure loads so every later DMA's lane-arming partner
# (DMA #n-8) has long completed -> no arming stalls
PF = 16

# test.py pokes these for profiling
TRACE = False
LAST_RESULT = None

_CACHE = {}


def build_nc(rows=ROWS):
    """Emit the Bass/Tile IR for one core handling `rows` rows."""
    n_it = rows // (SC * P)
    pf = min(PF, n_it)
    nc = bacc.Bacc("TRN2", target_bir_lowering=False, debug=False)
    msg = nc.dram_tensor("msg", [rows, MSG], mybir.dt.float32, kind="ExternalInput")
    gp = nc.dram_tensor("gp", [P, KCH * NPAR], mybir.dt.bfloat16, kind="ExternalInput")
    out = nc.dram_tensor(
        "out", [rows, MSG + NPAR], mybir.dt.float32, kind="ExternalOutput"
    )

    msg3 = msg[:, :].rearrange("(s c p) k -> s c p k", c=SC, p=P)
    out3 = out[:, :].rearrange("(s c p) k -> s c p k", c=SC, p=P)

    with tile.TileContext(nc) as tc:
        with (
            # every buffer's full live range fits in its pool, so the Tile
            # scheduler never emits slot-release relay waits (those ride
            # engine queues and serialize the pipeline cross-superchunk)
            tc.tile_pool(name="gpool", bufs=1) as gpool,
            tc.tile_pool(name="apool", bufs=n_it + 1) as apool,
            tc.tile_pool(name="bfpool", bufs=4) as bfpool,
            tc.tile_pool(name="bpool", bufs=4) as bpool,
            tc.tile_pool(name="parpool", bufs=n_it) as parpool,
            tc.tile_pool(name="ipool", bufs=4) as ipool,
            tc.tile_pool(name="ppool", bufs=8, space="PSUM") as ppool,
        ):
            # Gp resident in SBUF: gsb[q, kb*256 + n] = Gp_padded[kb*128 + q, n]
            # (loaded on the otherwise-idle SWDGE queue so the sync HWDGE
            # ring starts draining load(0) immediately)
            gsb = gpool.tile([P, KCH * NPAR], mybir.dt.bfloat16)
            nc.gpsimd.dma_start(out=gsb[:, :], in_=gp[:, :])

            a_t = {}
            abf_t = {}
            b_t = {}
            acc_t = {}

            def emit_load(it):
                a = apool.tile([P, SC, MSG], mybir.dt.float32, tag="a")
                nc.sync.dma_start(
                    out=a[:, :, :], in_=msg3[it, :, :, :].rearrange("c p k -> p c k")
                )
                a_t[it] = a

            def emit_msg_store(it):
                nc.sync.dma_start(
                    out=out3[it, :, :, 0:MSG].rearrange("c p k -> p c k"),
                    in_=a_t.pop(it)[:, :, :],
                )

            def emit_cast(it):
                abf = bfpool.tile([P, SC, KPAD], mybir.dt.bfloat16, tag="abf")
                nc.vector.memset(abf[:, :, MSG:KPAD], 0)
                nc.vector.tensor_copy(abf[:, :, 0:MSG], a_t[it][:, :, :])
                abf_t[it] = abf

            def emit_tr(it):
                # one xbar transpose for the whole superchunk, on the ACT
                # HWDGE ring (all TRs on ONE ring: concurrent xbar
                # transposes from two rings corrupt each other)
                b = bpool.tile([P, SC * KCH, P], mybir.dt.bfloat16, tag="b")
                nc.scalar.dma_start(
                    out=b[:, :, :],
                    in_=abf_t.pop(it)[:, :, :].rearrange("p c k -> p (c k)"),
                    transpose=True,
                )
                b_t[it] = b

            def emit_mm(it):
                b = b_t.pop(it)
                acc = ppool.tile([P, SC * NPAR], mybir.dt.float32, tag="acc")
                for c in range(SC):
                    for kb in range(KCH):
                        kk = P if kb < KCH - 1 else MSG - (KCH - 1) * P  # 104 tail
                        nc.tensor.matmul(
                            acc[:, c * NPAR : (c + 1) * NPAR],
                            b[0:kk, c * KCH + kb, :],
                            gsb[0:kk, kb * NPAR : (kb + 1) * NPAR],
                            start=(kb == 0),
                            stop=(kb == KCH - 1),
                        )
                acc_t[it] = acc

            def emit_parity(it):
                # mod 2 on DVE only: psum f32 -> i32 (numeric cast, exact
                # for integer sums <= 1000), AND 1, i32 -> f32
                ci = ipool.tile([P, SC * NPAR], mybir.dt.int32, tag="ci")
                nc.vector.tensor_copy(ci[:, :], acc_t.pop(it)[:, :])
                nc.vector.tensor_scalar(
                    ci[:, :], ci[:, :], 1, None, mybir.AluOpType.bitwise_and
                )
                par = parpool.tile([P, SC, NPAR], mybir.dt.float32, tag="par")
                nc.vector.tensor_copy(
                    par[:, :, :].rearrange("p c n -> p (c n)"), ci[:, :]
                )
                return par

            def emit_par_store(it, par):
                nc.gpsimd.dma_start(
                    out=out3[it, :, :, MSG : MSG + NPAR].rearrange("c p k -> p c k"),
                    in_=par[:, :, :],
                )

            for i in range(pf):
                emit_load(i)
            emit_cast(0)
            if n_it > 1:
                emit_cast(1)
            emit_tr(0)
            for it in range(n_it):
                if it + pf < n_it:
                    emit_load(it + pf)
                emit_msg_store(it)
                if it + 2 < n_it:
                    emit_cast(it + 2)
                if it + 1 < n_it:
                    emit_tr(it + 1)
                emit_mm(it)
                par = emit_parity(it)
                emit_par_store(it, par)

    nc.compile()
    return nc


def prep_gp(Gp):
    """Pad Gp to 1024 rows and swizzle to the [128, 8*256] bf16 SBUF layout."""
    gp = np.asarray(Gp, dtype=np.float32)
    gp_pad = np.zeros((KPAD, NPAR), dtype=np.float32)
    gp_pad[:MSG] = gp
    gsw = gp_pad.reshape(KCH, P, NPAR).transpose(1, 0, 2).reshape(P, KCH * NPAR)
    return np.ascontiguousarray(gsw).astype(ml_dtypes.bfloat16)


def kernel(message_bits, Gp):
    global LAST_RESULT
    msg = np.ascontiguousarray(np.asarray(message_bits, dtype=np.float32))
    assert msg.shape == (BATCH, MSG), msg.shape
    gsw = prep_gp(Gp)

    if "nc" not in _CACHE:
        _CACHE["nc"] = build_nc()
    nc = _CACHE["nc"]

    in_maps = [
        {"msg": msg[i * ROWS : (i + 1) * ROWS], "gp": gsw} for i in range(NCORES)
    ]
    res = run_bass_kernel_spmd(
        nc, in_maps, core_ids=list(range(NCORES)), trace=TRACE
    )
    LAST_RESULT = res
    return np.concatenate([r["out"] for r in res.results], axis=0)
